# revision 21
# baseline (speedup 1.0000x reference)
"""Causal GQA attention on 8 TRN2 NeuronCores.

Problem: q [2048, 32, 128] f32, k/v [2048, 8, 128] f32, causal attention
with 4 query heads per kv head (GQA). Sharding: tensor-parallel over kv
heads -- core i gets kv head i plus query heads 4i..4i+3. No cross-core
communication needed.

Per-core algorithm (T=S=2048, HQ=4 local q heads, D=128):
  * Q/K/V loaded as f32 (HWDGE), cast to fp16 (q on GPSIMD, k/v on DVE),
    K and Q transposed on the TensorE into [d, s] / [d, q] layouts so the
    QK^T contraction (over d) runs with d on partitions.
  * Scores TRANSPOSED: st[s_block=128, q_chunk<=512] = K_b^T-stationary
    x Q^T-moving; fp32 PSUM.
  * exp() on ScalarE reads PSUM scores (scale=1/sqrt(D) folded in),
    writes fp16 probabilities to SBUF. No max-subtraction needed.
  * Causal mask: GPSIMD affine_select zeroes the s>q triangle of the
    diagonal prob tiles after exp.
  * PV: prob block [s,q-tile] STATIONARY, moving operand [V_b | ones]
    [s, 129] fp16: accumulates [q, 128 out + 1 denom] in PSUM over s
    blocks -- softmax denominator for free.
  * Finalize: DVE reciprocal of denom + per-partition scalar multiply,
    DMA out per 2-tile half-chunk (tail-latency friendly).
  * PSUM budget (8 banks, accumulation groups are BANK-granular): sc pool
    2x[128,1024]f32 (4 banks) + pv pool 4x[128,132]f32 (4 banks, slots
    bank-padded). Transpose staging borrows the sc ring in parity-pairs
    so its WAR lands on a fast DVE copy, never on a pending finalize.
  * Software pipeline with flush depth 2: PV/finalize of pair i are
    emitted after QK of pairs i+1 AND i+2, so the exp(i) latency chain
    (sem + ~1.1us ScalarE + sem) hides behind two pairs of PE work.
  * Exact causal trim: each pair's second block computes only its valid
    columns, packed adjacent to block 0's span so a single contiguous
    exp covers both ([joff0, 2*chunk - j1*128)).
  * Snake schedule (even heads ascend chunks, odd heads descend): the
    first chunk needs only one kT group (fast start) and the last chunk
    is the smallest (short drain tail). All input DMAs are issued
    up-front in need order; identity transposes at kernel start keep the
    PE active during the DMA wait (HAM clock-gate warm-up); a dummy
    activation preloads the Exp table before the first real exp.
"""

import math

import numpy as np

import concourse.bass as bass
import concourse.tile as tile
from concourse import bacc, mybir
from concourse.masks import make_identity

P = 128
F32 = mybir.dt.float32
F16 = mybir.dt.float16
EXP = mybir.ActivationFunctionType.Exp

# Full problem shape (hardcoded; harness passes full unsharded inputs).
T_FULL = 2048
S_FULL = 2048
NH = 32
NKV = 8
D = 128
HQ = NH // NKV  # q heads per kv head (= per core)
N_CORES = 8


def _attention_body(tc, T, S, HQ, D, chunk):
    nc = tc.nc
    NT = T // P          # q tiles
    NB = S // P          # s blocks
    TPC = chunk // P     # q tiles per chunk
    NCH = T // chunk     # chunks
    assert TPC % 2 == 0 and T % chunk == 0 and S == T
    SCALE = 1.0 / math.sqrt(D)

    q = nc.dram_tensor("q", [T, HQ, D], F32, kind="ExternalInput").ap()
    k = nc.dram_tensor("k", [S, D], F32, kind="ExternalInput").ap()
    v = nc.dram_tensor("v", [S, D], F32, kind="ExternalInput").ap()
    out = nc.dram_tensor("out", [T, HQ, D], F32, kind="ExternalOutput").ap()

    from contextlib import ExitStack

    with ExitStack() as ctx:
        consts = ctx.enter_context(tc.tile_pool(name="consts", bufs=1))
        qT_pool = ctx.enter_context(tc.tile_pool(name="qT", bufs=2))
        et_pool = ctx.enter_context(tc.tile_pool(name="et", bufs=8))
        osb_pool = ctx.enter_context(tc.tile_pool(name="osb", bufs=4))
        rec_pool = ctx.enter_context(tc.tile_pool(name="rec", bufs=16))
        # PSUM: sc 2x2 banks, pv 4x1 (baseline layout; tp borrows pv slots)
        sc_psum = ctx.enter_context(tc.tile_pool(name="sc", bufs=2, space="PSUM"))
        pv_psum = ctx.enter_context(tc.tile_pool(name="pv", bufs=4, space="PSUM"))

        ident = consts.tile([P, P], F16)
        make_identity(nc, ident)
        # pull the Exp activation-table load off the first real exp
        act_warm = consts.tile([P, 1], F16, name="act_warm")
        nc.scalar.activation(act_warm, ident[:, 0:1], EXP)

        # ---- front-loaded input DMAs (all of k, q, v) ----
        k_nat32 = consts.tile([P, NB, P], F32)
        k_nat = consts.tile([P, NB, P], F16)
        k_r = k.rearrange("(b p) d -> p b d", p=P)
        q32 = consts.tile([P, HQ, NT, P], F32)
        q_rr = q.rearrange("(t p) h d -> p h t d", p=P)
        v_nat32 = consts.tile([P, NB, P], F32)
        v_r = v.rearrange("(b p) d -> p b d", p=P)

        # Coarse input DMAs in need order: HWDGE issue costs ~1.2us of
        # engine time per dma_start, so fewer+bigger wins the startup.
        nc.sync.dma_start(out=q32[:, 0, 0:8, :], in_=q_rr[:, 0, 0:8, :])
        nc.sync.dma_start(out=k_nat32[:, 0:4, :], in_=k_r[:, 0:4, :])
        nc.sync.dma_start(out=v_nat32[:, 0:8, :], in_=v_r[:, 0:8, :])
        nc.sync.dma_start(out=k_nat32[:, 4:16, :], in_=k_r[:, 4:16, :])
        nc.sync.dma_start(out=q32[:, 0, 8:16, :], in_=q_rr[:, 0, 8:16, :])
        nc.sync.dma_start(out=q32[:, 1, :, :], in_=q_rr[:, 1, :, :])
        nc.sync.dma_start(out=v_nat32[:, 8:16, :], in_=v_r[:, 8:16, :])
        nc.sync.dma_start(out=q32[:, 2, :, :], in_=q_rr[:, 2, :, :])
        nc.sync.dma_start(out=q32[:, 3, :, :], in_=q_rr[:, 3, :, :])

        # ---- HAM pre-warm: dummy N=512 matmuls keep the PE streaming
        # (serially) while input DMAs land, so the HAM clock gate opens
        # before the real work starts.
        warm = pv_psum.tile([P, P], F16, name="warm", tag="pv")
        for _ in range(24):
            nc.tensor.transpose(warm, ident, ident)

        # ---- casts ----
        def emit_k_cast(g):
            bg = 4 * g
            nc.vector.tensor_copy(
                k_nat[:, bg : bg + 4, :], k_nat32[:, bg : bg + 4, :]
            )

        for g in range(NB // 4):
            emit_k_cast(g)

        kT = consts.tile([P, NB * P], F16)

        def tp_pair(width):
            a = sc_psum.tile([P, width], F16, name=None, tag="sc")
            b = sc_psum.tile([P, width], F16, name=None, tag="sc")
            return a, b

        def emit_ktp(g):
            bg = 4 * g
            tpa, tpb = tp_pair(2 * P)
            for j in range(4):
                dst = (tpa, tpb)[j // 2]
                nc.tensor.transpose(
                    dst[:, (j % 2) * P : (j % 2 + 1) * P], k_nat[:, bg + j, :], ident
                )
            nc.vector.tensor_copy(kT[:, bg * P : (bg + 2) * P], tpa)
            nc.vector.tensor_copy(kT[:, (bg + 2) * P : (bg + 4) * P], tpb)

        emit_ktp(0)

        # ---- Q staging: cast on GPSIMD, PE transpose via sc-ring pairs ----
        q_nats = []
        q_cast_done = set()
        for h in range(HQ):
            qn = consts.tile([P, NT, P], F16, name=f"q_nat{h}", tag=f"q_nat{h}")
            q_nats.append(qn)

        def emit_q_cast(h, c):
            if (h, c) in q_cast_done:
                return
            q_cast_done.add((h, c))
            nc.vector.tensor_copy(
                q_nats[h][:, c * TPC : (c + 1) * TPC, :],
                q32[:, h, c * TPC : (c + 1) * TPC, :],
            )

        emit_q_cast(0, 0)

        qTs = {}

        def emit_qT_chunk(h, c):
            if h not in qTs:
                qTs[h] = qT_pool.tile([P, T], F16, name=f"qT{h}", tag="qT")
            qT = qTs[h]
            half = chunk // 2
            tpa, tpb = tp_pair(half)
            for j in range(TPC):
                dst = tpa if j < TPC // 2 else tpb
                jj = j % (TPC // 2)
                nc.tensor.transpose(
                    dst[:, jj * P : (jj + 1) * P], q_nats[h][:, c * TPC + j, :], ident
                )
            nc.vector.tensor_copy(qT[:, c * chunk : c * chunk + half], tpa)
            nc.vector.tensor_copy(qT[:, c * chunk + half : (c + 1) * chunk], tpb)

        emit_qT_chunk(0, 0)

        # ---- V: cast + ones column ----
        v_sb = consts.tile([P, NB, P + 1], F16)  # [s_in_block, b, d|ones]
        for bg in range(0, NB, 4):
            nc.vector.tensor_copy(
                v_sb[:, bg : bg + 4, 0:P], v_nat32[:, bg : bg + 4, :]
            )
        nc.vector.memset(v_sb[:, :, P : P + 1], 1.0)

        schedule = []
        for h in range(HQ):
            cs = range(NCH) if h % 2 == 0 else range(NCH - 1, -1, -1)
            for cc in cs:
                schedule.append((h, cc))

        qT_done = {(0, 0)}
        k_groups_done = {0}

        def emit_deps(h, c):
            for g in range(NB // 4):
                if g <= c and g not in k_groups_done:
                    k_groups_done.add(g)
                    emit_ktp(g)
            if (h, c) not in qT_done:
                qT_done.add((h, c))
                emit_q_cast(h, c)
                emit_qT_chunk(h, c)

        def emit_prefetch(idx):
            if idx + 1 < len(schedule):
                emit_deps(*schedule[idx + 1])

        chunk_state = {}

        def get_state(idx, h, c):
            if idx not in chunk_state:
                chunk_state[idx] = {
                    "pvs": [
                        pv_psum.tile(
                            [P, 132], F32, name=f"pv{idx}_{i}", tag="pv"
                        )
                        for i in range(TPC)
                    ],
                    "osb": osb_pool.tile(
                        [P, TPC, P], F32, name=f"osb{idx}", tag="osb"
                    ),
                }
            return chunk_state[idx]

        def emit_qk(idx, h, c, b0):
            qT = qTs[h]
            sc = sc_psum.tile([P, 2 * chunk], F32, name=f"sc{idx}_{b0}", tag="sc")
            joff0 = max(0, b0 - c * TPC) * P
            j1 = max(0, b0 + 1 - c * TPC)
            nc.tensor.matmul(
                sc[:, joff0:chunk],
                lhsT=kT[:, b0 * P : (b0 + 1) * P],
                rhs=qT[:, c * chunk + joff0 : (c + 1) * chunk],
                start=True,
                stop=True,
            )
            # block 1 is packed at [chunk, 2*chunk - j1*P): its own causal
            # start, placed adjacent so one exp covers a contiguous span
            nc.tensor.matmul(
                sc[:, chunk : 2 * chunk - j1 * P],
                lhsT=kT[:, (b0 + 1) * P : (b0 + 2) * P],
                rhs=qT[:, c * chunk + j1 * P : (c + 1) * chunk],
                start=True,
                stop=True,
            )
            return sc

        def emit_exp_mask(idx, h, c, b0, sc):
            et = et_pool.tile([P, 2 * chunk], F16, name=f"et{idx}_{b0}", tag="et")
            j1 = max(0, b0 + 1 - c * TPC)
            hi = 2 * chunk - j1 * P
            if b0 >= c * TPC:
                joff0 = (b0 - c * TPC) * P
                nc.scalar.activation(
                    et[:, joff0:hi], sc[:, joff0:hi], EXP, scale=SCALE
                )
                # diagonal tiles: block0 at col j0*P, block1 at col chunk
                for dsl in (
                    et[:, joff0 : joff0 + P],
                    et[:, chunk : chunk + P],
                ):
                    nc.gpsimd.affine_select(
                        out=dsl,
                        in_=dsl,
                        pattern=[[1, P]],
                        compare_op=mybir.AluOpType.is_ge,
                        fill=0.0,
                        base=0,
                        channel_multiplier=-1,
                    )
            else:
                nc.scalar.activation(et[:, 0:hi], sc[:, 0:hi], EXP, scale=SCALE)
            return et

        def emit_pv(idx, h, c, b0, et):
            st = get_state(idx, h, c)
            j1 = max(0, b0 + 1 - c * TPC)
            work = []
            for i, b in enumerate((b0, b0 + 1)):
                j = b - c * TPC
                for tloc in range(max(0, j), TPC):
                    work.append((i, b, tloc, tloc == j))
            work.sort(key=lambda w: w[3])  # diagonal-tile PV last
            for i, b, tloc, _ in work:
                t = c * TPC + tloc
                col = tloc * P if i == 0 else chunk + (tloc - j1) * P
                nc.tensor.matmul(
                    st["pvs"][tloc][:, 0 : P + 1],
                    lhsT=et[:, col : col + P],
                    rhs=v_sb[:, b, :],
                    start=(b == 0),
                    stop=(b == t),
                )

        def emit_finalize_dma(idx, h, c, b0):
            """Finalize + DMA the two diagonal tiles of this pair."""
            st = chunk_state[idx]
            lo = b0 - c * TPC
            if lo < 0:
                return
            for tloc in (lo, lo + 1):
                pv = st["pvs"][tloc][:, 0 : P + 1]
                rec = rec_pool.tile(
                    [P, 1], F32, name=f"rec{idx}_{tloc}", tag="rec"
                )
                nc.vector.reciprocal(rec, pv[:, P : P + 1])
                nc.vector.tensor_scalar_mul(
                    st["osb"][:, tloc, :], pv[:, 0:P], rec
                )
            nc.sync.dma_start(
                out=out[
                    c * chunk + lo * P : c * chunk + (lo + 2) * P, h, :
                ].rearrange("(t p) d -> p t d", p=P),
                in_=st["osb"][:, lo : lo + 2, :],
            )

        def flush(entry):
            idx, h, c, b0, last, et = entry
            emit_pv(idx, h, c, b0, et)
            emit_finalize_dma(idx, h, c, b0)
            if last:
                del chunk_state[idx]

        # one flat software-pipelined stream over every (chunk, pair)
        stream = []
        for idx, (h, c) in enumerate(schedule):
            nblocks = TPC * (c + 1)
            for b0 in range(0, nblocks, 2):
                stream.append((idx, h, c, b0, b0 == nblocks - 2))

        pending = []
        first = True
        for idx, h, c, b0, last in stream:
            if b0 == (2 if first else 0):
                first = False
                emit_prefetch(idx)
            sc = emit_qk(idx, h, c, b0)
            if len(pending) == 2:
                flush(pending.pop(0))
            et = emit_exp_mask(idx, h, c, b0, sc)
            pending.append((idx, h, c, b0, last, et))
        for e in pending:
            flush(e)


def build_nc(T=T_FULL, S=S_FULL, HQ=HQ, D=D, chunk=512):
    nc = bacc.Bacc(
        "TRN2", target_bir_lowering=False, debug=False, enable_asserts=False
    )
    with tile.TileContext(nc) as tc:
        _attention_body(tc, T, S, HQ, D, chunk)
    nc.compile()
    return nc


_NC_CACHE = {}


def _get_nc():
    if "nc" not in _NC_CACHE:
        _NC_CACHE["nc"] = build_nc()
    return _NC_CACHE["nc"]


def kernel(q, k, v):
    """Full-problem entry point: q [2048,32,128], k/v [2048,8,128] f32."""
    from concourse.bass_utils import run_bass_kernel_spmd

    q = np.asarray(q, dtype=np.float32)
    k = np.asarray(k, dtype=np.float32)
    v = np.asarray(v, dtype=np.float32)

    nc = _get_nc()
    in_maps = []
    for i in range(N_CORES):
        in_maps.append(
            {
                "q": np.ascontiguousarray(q[:, HQ * i : HQ * (i + 1), :]),
                "k": np.ascontiguousarray(k[:, i, :]),
                "v": np.ascontiguousarray(v[:, i, :]),
            }
        )
    res = run_bass_kernel_spmd(nc, in_maps, core_ids=list(range(N_CORES)))
    out = np.empty((T_FULL, NH, D), dtype=np.float32)
    for i in range(N_CORES):
        out[:, HQ * i : HQ * (i + 1), :] = res.results[i]["out"]
    return out


# revision 22
# speedup vs baseline: 1.0118x; 1.0118x over previous
"""Causal GQA attention on 8 TRN2 NeuronCores.

Problem: q [2048, 32, 128] f32, k/v [2048, 8, 128] f32, causal attention
with 4 query heads per kv head (GQA). Sharding: tensor-parallel over kv
heads -- core i gets kv head i plus query heads 4i..4i+3. No cross-core
communication needed.

Per-core algorithm (T=S=2048, HQ=4 local q heads, D=128):
  * Q/K/V loaded as f32 (HWDGE), cast to fp16 (q on GPSIMD, k/v on DVE),
    K and Q transposed on the TensorE into [d, s] / [d, q] layouts so the
    QK^T contraction (over d) runs with d on partitions.
  * Scores TRANSPOSED: st[s_block=128, q_chunk<=512] = K_b^T-stationary
    x Q^T-moving; fp32 PSUM.
  * exp() on ScalarE reads PSUM scores (scale=1/sqrt(D) folded in),
    writes fp16 probabilities to SBUF. No max-subtraction needed.
  * Causal mask: GPSIMD affine_select zeroes the s>q triangle of the
    diagonal prob tiles after exp.
  * PV: prob block [s,q-tile] STATIONARY, moving operand [V_b | ones]
    [s, 129] fp16: accumulates [q, 128 out + 1 denom] in PSUM over s
    blocks -- softmax denominator for free.
  * Finalize: DVE reciprocal of denom + per-partition scalar multiply,
    DMA out per 2-tile half-chunk (tail-latency friendly).
  * PSUM budget (8 banks, accumulation groups are BANK-granular): sc pool
    2x[128,1024]f32 (4 banks) + pv pool 4x[128,132]f32 (4 banks, slots
    bank-padded). Transpose staging borrows the sc ring in parity-pairs
    so its WAR lands on a fast DVE copy, never on a pending finalize.
  * Software pipeline with flush depth 2: PV/finalize of pair i are
    emitted after QK of pairs i+1 AND i+2, so the exp(i) latency chain
    (sem + ~1.1us ScalarE + sem) hides behind two pairs of PE work.
  * Exact causal trim: each pair's second block computes only its valid
    columns, packed adjacent to block 0's span so a single contiguous
    exp covers both ([joff0, 2*chunk - j1*128)).
  * Snake schedule (even heads ascend chunks, odd heads descend): the
    first chunk needs only one kT group (fast start) and the last chunk
    is the smallest (short drain tail). All input DMAs are issued
    up-front in need order; identity transposes at kernel start keep the
    PE active during the DMA wait (HAM clock-gate warm-up); a dummy
    activation preloads the Exp table before the first real exp.
"""

import math

import numpy as np

import concourse.bass as bass
import concourse.tile as tile
from concourse import bacc, mybir
from concourse.masks import make_identity

P = 128
F32 = mybir.dt.float32
F16 = mybir.dt.float16
EXP = mybir.ActivationFunctionType.Exp

# Full problem shape (hardcoded; harness passes full unsharded inputs).
T_FULL = 2048
S_FULL = 2048
NH = 32
NKV = 8
D = 128
HQ = NH // NKV  # q heads per kv head (= per core)
N_CORES = 8


def _attention_body(tc, T, S, HQ, D, chunk):
    nc = tc.nc
    NT = T // P          # q tiles
    NB = S // P          # s blocks
    TPC = chunk // P     # q tiles per chunk
    NCH = T // chunk     # chunks
    assert TPC % 2 == 0 and T % chunk == 0 and S == T
    SCALE = 1.0 / math.sqrt(D)

    q = nc.dram_tensor("q", [T, HQ, D], F32, kind="ExternalInput").ap()
    k = nc.dram_tensor("k", [S, D], F32, kind="ExternalInput").ap()
    v = nc.dram_tensor("v", [S, D], F32, kind="ExternalInput").ap()
    out = nc.dram_tensor("out", [T, HQ, D], F32, kind="ExternalOutput").ap()

    from contextlib import ExitStack

    with ExitStack() as ctx:
        consts = ctx.enter_context(tc.tile_pool(name="consts", bufs=1))
        qT_pool = ctx.enter_context(tc.tile_pool(name="qT", bufs=3))
        et_pool = ctx.enter_context(tc.tile_pool(name="et", bufs=8))
        osb_pool = ctx.enter_context(tc.tile_pool(name="osb", bufs=4))
        rec_pool = ctx.enter_context(tc.tile_pool(name="rec", bufs=16))
        # PSUM: sc 2x2 banks, pv 4x1 (baseline layout; tp borrows pv slots)
        sc_psum = ctx.enter_context(tc.tile_pool(name="sc", bufs=2, space="PSUM"))
        pv_psum = ctx.enter_context(tc.tile_pool(name="pv", bufs=4, space="PSUM"))

        ident = consts.tile([P, P], F16)
        make_identity(nc, ident)
        # pull the Exp activation-table load off the first real exp
        act_warm = consts.tile([P, 1], F16, name="act_warm")
        nc.scalar.activation(act_warm, ident[:, 0:1], EXP)

        # ---- front-loaded input DMAs (all of k, q, v) ----
        k_nat32 = consts.tile([P, NB, P], F32)
        k_nat = consts.tile([P, NB, P], F16)
        k_r = k.rearrange("(b p) d -> p b d", p=P)
        for bg in range(0, NB, 4):
            nc.sync.dma_start(
                out=k_nat32[:, bg : bg + 4, :], in_=k_r[:, bg : bg + 4, :]
            )

        q32 = consts.tile([P, HQ, NT, P], F32)
        q_rr = q.rearrange("(t p) h d -> p h t d", p=P)

        def q_dma(h, c):
            nc.sync.dma_start(
                out=q32[:, h, c * TPC : (c + 1) * TPC, :],
                in_=q_rr[:, h, c * TPC : (c + 1) * TPC, :],
            )

        for c in range(NCH):
            q_dma(0, c)
        for c in range(NCH - 1, -1, -1):
            q_dma(1, c)

        v_nat32 = consts.tile([P, NB, P], F32)
        v_r = v.rearrange("(b p) d -> p b d", p=P)
        for bg in range(0, NB, 4):
            nc.sync.dma_start(
                out=v_nat32[:, bg : bg + 4, :], in_=v_r[:, bg : bg + 4, :]
            )
        for h in range(2, HQ):
            cs = range(NCH) if h % 2 == 0 else range(NCH - 1, -1, -1)
            for c in cs:
                q_dma(h, c)

        # ---- HAM pre-warm: dummy N=512 matmuls keep the PE streaming
        # (serially) while input DMAs land, so the HAM clock gate opens
        # before the real work starts.
        warm = pv_psum.tile([P, P], F16, name="warm", tag="pv")
        for _ in range(24):
            nc.tensor.transpose(warm, ident, ident)

        # ---- casts ----
        def emit_k_cast(g):
            bg = 4 * g
            nc.vector.tensor_copy(
                k_nat[:, bg : bg + 4, :], k_nat32[:, bg : bg + 4, :]
            )

        for g in range(NB // 4):
            emit_k_cast(g)

        kT = consts.tile([P, NB * P], F16)

        def tp_pair(width):
            a = sc_psum.tile([P, width], F16, name=None, tag="sc")
            b = sc_psum.tile([P, width], F16, name=None, tag="sc")
            return a, b

        def emit_ktp(g):
            bg = 4 * g
            tpa, tpb = tp_pair(2 * P)
            for j in range(4):
                dst = (tpa, tpb)[j // 2]
                nc.tensor.transpose(
                    dst[:, (j % 2) * P : (j % 2 + 1) * P], k_nat[:, bg + j, :], ident
                )
            nc.vector.tensor_copy(kT[:, bg * P : (bg + 2) * P], tpa)
            nc.vector.tensor_copy(kT[:, (bg + 2) * P : (bg + 4) * P], tpb)

        emit_ktp(0)

        # ---- Q staging: cast on GPSIMD, PE transpose via sc-ring pairs ----
        q_nats = []
        q_cast_done = set()
        for h in range(HQ):
            qn = consts.tile([P, NT, P], F16, name=f"q_nat{h}", tag=f"q_nat{h}")
            q_nats.append(qn)

        def emit_q_cast(h, c):
            if (h, c) in q_cast_done:
                return
            q_cast_done.add((h, c))
            nc.vector.tensor_copy(
                q_nats[h][:, c * TPC : (c + 1) * TPC, :],
                q32[:, h, c * TPC : (c + 1) * TPC, :],
            )

        emit_q_cast(0, 0)

        qTs = {}

        def emit_qT_chunk(h, c):
            if h not in qTs:
                qTs[h] = qT_pool.tile([P, T], F16, name=f"qT{h}", tag="qT")
            qT = qTs[h]
            half = chunk // 2
            tpa, tpb = tp_pair(half)
            for j in range(TPC):
                dst = tpa if j < TPC // 2 else tpb
                jj = j % (TPC // 2)
                nc.tensor.transpose(
                    dst[:, jj * P : (jj + 1) * P], q_nats[h][:, c * TPC + j, :], ident
                )
            nc.vector.tensor_copy(qT[:, c * chunk : c * chunk + half], tpa)
            nc.vector.tensor_copy(qT[:, c * chunk + half : (c + 1) * chunk], tpb)

        emit_qT_chunk(0, 0)

        # ---- V: cast + ones column ----
        v_sb = consts.tile([P, NB, P + 1], F16)  # [s_in_block, b, d|ones]
        for bg in range(0, NB, 4):
            nc.vector.tensor_copy(
                v_sb[:, bg : bg + 4, 0:P], v_nat32[:, bg : bg + 4, :]
            )
        nc.vector.memset(v_sb[:, :, P : P + 1], 1.0)

        schedule = []
        for h in range(HQ):
            cs = range(NCH) if h % 2 == 0 else range(NCH - 1, -1, -1)
            for cc in cs:
                schedule.append((h, cc))

        qT_done = {(0, 0)}
        k_groups_done = {0}

        def emit_deps(h, c):
            for g in range(NB // 4):
                if g <= c and g not in k_groups_done:
                    k_groups_done.add(g)
                    emit_ktp(g)
            if (h, c) not in qT_done:
                qT_done.add((h, c))
                emit_q_cast(h, c)
                emit_qT_chunk(h, c)

        def emit_prefetch(idx):
            # two chunks ahead: qT/casts land a full chunk early, so the
            # DVE copy is never on the next chunk's critical path
            for ahead in (1, 2):
                if idx + ahead < len(schedule):
                    emit_deps(*schedule[idx + ahead])

        chunk_state = {}

        def get_state(idx, h, c):
            if idx not in chunk_state:
                chunk_state[idx] = {
                    "pvs": [
                        pv_psum.tile(
                            [P, 132], F32, name=f"pv{idx}_{i}", tag="pv"
                        )
                        for i in range(TPC)
                    ],
                    "osb": osb_pool.tile(
                        [P, TPC, P], F32, name=f"osb{idx}", tag="osb"
                    ),
                }
            return chunk_state[idx]

        def emit_qk(idx, h, c, b0):
            qT = qTs[h]
            sc = sc_psum.tile([P, 2 * chunk], F32, name=f"sc{idx}_{b0}", tag="sc")
            joff0 = max(0, b0 - c * TPC) * P
            j1 = max(0, b0 + 1 - c * TPC)
            nc.tensor.matmul(
                sc[:, joff0:chunk],
                lhsT=kT[:, b0 * P : (b0 + 1) * P],
                rhs=qT[:, c * chunk + joff0 : (c + 1) * chunk],
                start=True,
                stop=True,
            )
            # block 1 is packed at [chunk, 2*chunk - j1*P): its own causal
            # start, placed adjacent so one exp covers a contiguous span
            nc.tensor.matmul(
                sc[:, chunk : 2 * chunk - j1 * P],
                lhsT=kT[:, (b0 + 1) * P : (b0 + 2) * P],
                rhs=qT[:, c * chunk + j1 * P : (c + 1) * chunk],
                start=True,
                stop=True,
            )
            return sc

        def emit_exp_mask(idx, h, c, b0, sc):
            et = et_pool.tile([P, 2 * chunk], F16, name=f"et{idx}_{b0}", tag="et")
            j1 = max(0, b0 + 1 - c * TPC)
            hi = 2 * chunk - j1 * P
            if b0 >= c * TPC:
                joff0 = (b0 - c * TPC) * P
                nc.scalar.activation(
                    et[:, joff0:hi], sc[:, joff0:hi], EXP, scale=SCALE
                )
                # diagonal tiles: block0 at col j0*P, block1 at col chunk
                for dsl in (
                    et[:, joff0 : joff0 + P],
                    et[:, chunk : chunk + P],
                ):
                    nc.gpsimd.affine_select(
                        out=dsl,
                        in_=dsl,
                        pattern=[[1, P]],
                        compare_op=mybir.AluOpType.is_ge,
                        fill=0.0,
                        base=0,
                        channel_multiplier=-1,
                    )
            else:
                nc.scalar.activation(et[:, 0:hi], sc[:, 0:hi], EXP, scale=SCALE)
            return et

        def emit_pv(idx, h, c, b0, et):
            st = get_state(idx, h, c)
            j1 = max(0, b0 + 1 - c * TPC)
            work = []
            for i, b in enumerate((b0, b0 + 1)):
                j = b - c * TPC
                for tloc in range(max(0, j), TPC):
                    work.append((i, b, tloc, tloc == j))
            work.sort(key=lambda w: w[3])  # diagonal-tile PV last
            for i, b, tloc, _ in work:
                t = c * TPC + tloc
                col = tloc * P if i == 0 else chunk + (tloc - j1) * P
                nc.tensor.matmul(
                    st["pvs"][tloc][:, 0 : P + 1],
                    lhsT=et[:, col : col + P],
                    rhs=v_sb[:, b, :],
                    start=(b == 0),
                    stop=(b == t),
                )

        def emit_finalize_dma(idx, h, c, b0):
            """Finalize + DMA the two diagonal tiles of this pair."""
            st = chunk_state[idx]
            lo = b0 - c * TPC
            if lo < 0:
                return
            for tloc in (lo, lo + 1):
                pv = st["pvs"][tloc][:, 0 : P + 1]
                rec = rec_pool.tile(
                    [P, 1], F32, name=f"rec{idx}_{tloc}", tag="rec"
                )
                nc.vector.reciprocal(rec, pv[:, P : P + 1])
                nc.vector.tensor_scalar_mul(
                    st["osb"][:, tloc, :], pv[:, 0:P], rec
                )
            nc.sync.dma_start(
                out=out[
                    c * chunk + lo * P : c * chunk + (lo + 2) * P, h, :
                ].rearrange("(t p) d -> p t d", p=P),
                in_=st["osb"][:, lo : lo + 2, :],
            )

        def flush(entry):
            idx, h, c, b0, last, et = entry
            emit_pv(idx, h, c, b0, et)
            emit_finalize_dma(idx, h, c, b0)
            if last:
                del chunk_state[idx]

        # one flat software-pipelined stream over every (chunk, pair)
        stream = []
        for idx, (h, c) in enumerate(schedule):
            nblocks = TPC * (c + 1)
            for b0 in range(0, nblocks, 2):
                stream.append((idx, h, c, b0, b0 == nblocks - 2))

        pending = []
        first = True
        for idx, h, c, b0, last in stream:
            if b0 == (2 if first else 0):
                first = False
                emit_prefetch(idx)
            sc = emit_qk(idx, h, c, b0)
            if len(pending) == 2:
                flush(pending.pop(0))
            et = emit_exp_mask(idx, h, c, b0, sc)
            pending.append((idx, h, c, b0, last, et))
        for e in pending:
            flush(e)


def build_nc(T=T_FULL, S=S_FULL, HQ=HQ, D=D, chunk=512):
    nc = bacc.Bacc(
        "TRN2", target_bir_lowering=False, debug=False, enable_asserts=False
    )
    with tile.TileContext(nc) as tc:
        _attention_body(tc, T, S, HQ, D, chunk)
    nc.compile()
    return nc


_NC_CACHE = {}


def _get_nc():
    if "nc" not in _NC_CACHE:
        _NC_CACHE["nc"] = build_nc()
    return _NC_CACHE["nc"]


def kernel(q, k, v):
    """Full-problem entry point: q [2048,32,128], k/v [2048,8,128] f32."""
    from concourse.bass_utils import run_bass_kernel_spmd

    q = np.asarray(q, dtype=np.float32)
    k = np.asarray(k, dtype=np.float32)
    v = np.asarray(v, dtype=np.float32)

    nc = _get_nc()
    in_maps = []
    for i in range(N_CORES):
        in_maps.append(
            {
                "q": np.ascontiguousarray(q[:, HQ * i : HQ * (i + 1), :]),
                "k": np.ascontiguousarray(k[:, i, :]),
                "v": np.ascontiguousarray(v[:, i, :]),
            }
        )
    res = run_bass_kernel_spmd(nc, in_maps, core_ids=list(range(N_CORES)))
    out = np.empty((T_FULL, NH, D), dtype=np.float32)
    for i in range(N_CORES):
        out[:, HQ * i : HQ * (i + 1), :] = res.results[i]["out"]
    return out
